# revision 1
# baseline (speedup 1.0000x reference)
"""YOLOv1 loss kernel for Trainium2, 8-core data-parallel.

Strategy: shard batch (8192) across 8 cores (1024 each). Each core
processes its shard in NCHUNK chunks of G*128 batch items laid out as
[128 partitions, G, 1470] in SBUF (channel-major free dim). All box/IoU
arithmetic runs on the Vector engine (fused scalar_tensor_tensor ops
where possible); sqrt/square run on the Scalar (ACT) engine; DMAs on the
Sync (HWDGE) engine. Per-partition partial sums accumulate on-chip via
tensor_tensor_reduce; host sums the 8x128x3 partials and divides by B.

IoU is computed in cell-relative scaled coordinates: all three boxes of
a cell share the same (+m, +n)/G offset, so IoU is invariant to it, and
invariant to a uniform x-scale. With half-extents h = 3.5*w the overlap
width is min(2*ha, 2*hb, ha+hb-|dcx|) clamped at 0 (units: 7*w), and
union = 49*(wa*ha_frac...) i.e. 49*(w_a*h_a + w_g*h_g) - inter.
"""

import sys

import numpy as np

for _p in ("/opt/trn_rl_repo", "/root/.axon_site/_ro/trn_rl_repo"):
    if _p not in sys.path:
        sys.path.insert(0, _p)

import concourse.bass as bass
import concourse.mybir as mybir
from concourse.bass_utils import run_bass_kernel_spmd

F32 = mybir.dt.float32
U32 = mybir.dt.uint32
Alu = mybir.AluOpType
Act = mybir.ActivationFunctionType

B_TOTAL = 8192
NCORES = 8
B_CORE = B_TOTAL // NCORES  # 1024
P = 128
G = 2  # batch groups folded into the free dim per chunk
CHUNK = P * G  # 256
NCHUNK = B_CORE // CHUNK  # 4
C = 30
CELLS = 49
ROW = C * CELLS  # 1470


def build_nc(g: int = G, nchunk: int = NCHUNK):
    chunk = P * g
    nc = bass.Bass()
    pred = nc.declare_dram_parameter("pred", [B_CORE, ROW], F32, isOutput=False)
    labels = nc.declare_dram_parameter("labels", [B_CORE, ROW], F32, isOutput=False)
    out = nc.declare_dram_parameter("out", [P, 4], F32, isOutput=True)

    fshape = [P, g, CELLS]
    bshape = [P, g, 20 * CELLS]

    _ctr = [0]

    def sb(shape):
        _ctr[0] += 1
        return ctx_stack.enter_context(
            nc.sbuf_tensor(f"t{_ctr[0]}", shape, F32)
        )

    from contextlib import ExitStack

    ctx_stack = ExitStack()
    with ctx_stack:
        # double-buffered input tiles
        pt = [sb([P, g, ROW]) for _ in range(2)]
        lt = [sb([P, g, ROW]) for _ in range(2)]
        # ACT outputs (single-buffered; protected by sem schedule)
        sp2, sl2, sp3, sl3 = sb(fshape), sb(fshape), sb(fshape), sb(fshape)
        sp7, sl7, sp8, sl8 = sb(fshape), sb(fshape), sb(fshape), sb(fshape)
        q4, q9 = sb(fshape), sb(fshape)
        qx1, qy1, qx2, qy2 = sb(fshape), sb(fshape), sb(fshape), sb(fshape)
        e1, e2 = sb(fshape), sb(fshape)
        qsw1, qsh1, qsw2, qsh2 = sb(fshape), sb(fshape), sb(fshape), sb(fshape)
        sqcls = sb(bshape)
        # DVE temps
        dx1, dy1, dx2, dy2 = sb(fshape), sb(fshape), sb(fshape), sb(fshape)
        dx2c, dy2c = sb(fshape), sb(fshape)
        adx1, ady1, adx2, ady2 = sb(fshape), sb(fshape), sb(fshape), sb(fshape)
        sw1, sh1, sw2, sh2 = sb(fshape), sb(fshape), sb(fshape), sb(fshape)
        ta1, tb1, tc1 = sb(fshape), sb(fshape), sb(fshape)
        ta2, tb2, tc2 = sb(fshape), sb(fshape), sb(fshape)
        ta3, tb3, tc3 = sb(fshape), sb(fshape), sb(fshape)
        ta4, tb4, tc4 = sb(fshape), sb(fshape), sb(fshape)
        iw1, ih1, iw2, ih2 = sb(fshape), sb(fshape), sb(fshape), sb(fshape)
        int1, int2 = sb(fshape), sb(fshape)
        a1, a2, ag = sb(fshape), sb(fshape), sb(fshape)
        s1, s2 = sb(fshape), sb(fshape)
        u1, u2 = sb(fshape), sb(fshape)
        r1, r2 = sb(fshape), sb(fshape)
        iou1, iou2 = sb(fshape), sb(fshape)
        use1, objm = sb(fshape), sb(fshape)
        d1, d2 = sb(fshape), sb(fshape)
        dcls = sb(bshape)
        jbig = sb(bshape)
        dsw1, dsh1, dsw2, dsh2 = sb(fshape), sb(fshape), sb(fshape), sb(fshape)
        c1a, c1b, c2a, c2b = sb(fshape), sb(fshape), sb(fshape), sb(fshape)
        coor1, coor2 = sb(fshape), sb(fshape)
        de, hde, dc = sb(fshape), sb(fshape), sb(fshape)
        nc2t, mix, tsel = sb(fshape), sb(fshape), sb(fshape)
        clsum = sb(fshape)
        base, base2, base3, dd = sb(fshape), sb(fshape), sb(fshape), sb(fshape)
        junk = sb(fshape)
        junk2 = sb(fshape)
        red0 = ctx_stack.enter_context(nc.sbuf_tensor("red0", [P, 1], F32))
        red1 = ctx_stack.enter_context(nc.sbuf_tensor("red1", [P, 1], F32))
        acc = ctx_stack.enter_context(nc.sbuf_tensor("acc", [P, 4], F32))

        dma_sem = ctx_stack.enter_context(nc.semaphore("dma_sem"))
        sA1 = ctx_stack.enter_context(nc.semaphore("sA1"))
        sA2 = ctx_stack.enter_context(nc.semaphore("sA2"))
        sD1 = ctx_stack.enter_context(nc.semaphore("sD1"))
        sD2 = ctx_stack.enter_context(nc.semaphore("sD2"))
        v_done = ctx_stack.enter_context(nc.semaphore("v_done"))
        sGP = ctx_stack.enter_context(nc.semaphore("sGP"))
        block = ctx_stack.enter_context(nc.Block())

        def ch(t, c):  # channel slice -> [P, g, 49]
            return t[:, :, c * CELLS:(c + 1) * CELLS]

        def cls_blk(t):  # channels 10..29 -> [P, g, 980]
            return t[:, :, 10 * CELLS:30 * CELLS]

        @block.sync
        def _(sync):
            for i in range(nchunk):
                s = i % 2
                if i >= 1:
                    sync.wait_ge(dma_sem, 32 * i)
                if i >= 2:
                    sync.wait_ge(v_done, i - 1)
                rows = slice(i * chunk, (i + 1) * chunk)
                sync.dma_start(
                    out=pt[s][:],
                    in_=pred[rows].rearrange("(g p) d -> p g d", p=P),
                ).then_inc(dma_sem, 16)
                sync.dma_start(
                    out=lt[s][:],
                    in_=labels[rows].rearrange("(g p) d -> p g d", p=P),
                ).then_inc(dma_sem, 16)
            sync.wait_ge(v_done, nchunk)
            sync.dma_start(out=out[:], in_=acc[:]).then_inc(dma_sem, 16)
            sync.wait_ge(dma_sem, 32 * nchunk + 16)

        @block.gpsimd
        def _(gp):
            for i in range(nchunk):
                s = i % 2
                if i >= 1:
                    gp.wait_ge(v_done, i)
                gp.wait_ge(dma_sem, 32 * (i + 1))
                p, l = pt[s], lt[s]
                gp.tensor_tensor(dx2c[:], ch(p, 5), ch(l, 5), Alu.subtract)
                gp.tensor_tensor(dy2c[:], ch(p, 6), ch(l, 6), Alu.subtract)
                gp.tensor_scalar(objm[:], ch(l, 4), 1.0, None, Alu.is_equal)
                gp.tensor_tensor(dcls[:], cls_blk(p), cls_blk(l), Alu.subtract)
                gp.drain().then_inc(sGP, 1)

        @block.scalar
        def _(act):
            for i in range(nchunk):
                s = i % 2
                if i >= 1:
                    act.wait_ge(v_done, i)
                act.wait_ge(dma_sem, 32 * (i + 1))
                p, l = pt[s], lt[s]
                # phase 1: sqrts of w/h channels + conf squares
                act.activation(sp2[:], ch(p, 2), Act.Sqrt)
                act.activation(sl2[:], ch(l, 2), Act.Sqrt)
                act.activation(sp3[:], ch(p, 3), Act.Sqrt)
                act.activation(sl3[:], ch(l, 3), Act.Sqrt)
                act.activation(sp7[:], ch(p, 7), Act.Sqrt)
                act.activation(sl7[:], ch(l, 7), Act.Sqrt)
                act.activation(sp8[:], ch(p, 8), Act.Sqrt)
                act.activation(sl8[:], ch(l, 8), Act.Sqrt)
                act.activation(q4[:], ch(p, 4), Act.Square)
                act.activation(q9[:], ch(p, 9), Act.Square)
                act.drain().then_inc(sA1, 1)
                # phase 2a: squares of DVE diffs
                act.wait_ge(sD1, i + 1)
                act.activation(qx1[:], dx1[:], Act.Square)
                act.activation(qy1[:], dy1[:], Act.Square)
                act.activation(e1[:], d1[:], Act.Square)
                act.activation(e2[:], d2[:], Act.Square)
                act.wait_ge(sGP, i + 1)
                act.activation(qx2[:], dx2c[:], Act.Square)
                act.activation(qy2[:], dy2c[:], Act.Square)
                act.activation(sqcls[:], dcls[:], Act.Square)
                # phase 2b: squares of sqrt diffs
                act.wait_ge(sD2, i + 1)
                act.activation(qsw1[:], dsw1[:], Act.Square)
                act.activation(qsh1[:], dsh1[:], Act.Square)
                act.activation(qsw2[:], dsw2[:], Act.Square)
                act.activation(qsh2[:], dsh2[:], Act.Square)
                act.drain().then_inc(sA2, 1)

        @block.vector
        def _(v):
            stt = v.scalar_tensor_tensor
            tt = v.tensor_tensor
            ts = v.tensor_scalar

            v.memset(acc[:], 0.0)
            v.drain()
            for i in range(nchunk):
                s = i % 2
                v.wait_ge(dma_sem, 32 * (i + 1))
                p, l = pt[s], lt[s]
                # --- wave 1: direct from inputs ---
                tt(dx1[:], ch(p, 0), ch(l, 0), Alu.subtract)
                tt(dy1[:], ch(p, 1), ch(l, 1), Alu.subtract)
                tt(dx2[:], ch(p, 5), ch(l, 0), Alu.subtract)
                tt(dy2[:], ch(p, 6), ch(l, 1), Alu.subtract)
                tt(sw1[:], ch(p, 2), ch(l, 2), Alu.add)
                tt(sh1[:], ch(p, 3), ch(l, 3), Alu.add)
                tt(sw2[:], ch(p, 7), ch(l, 2), Alu.add)
                tt(sh2[:], ch(p, 8), ch(l, 3), Alu.add)
                tt(tc1[:], ch(p, 2), ch(l, 2), Alu.min)
                tt(tc2[:], ch(p, 3), ch(l, 3), Alu.min)
                tt(tc3[:], ch(p, 7), ch(l, 2), Alu.min)
                tt(tc4[:], ch(p, 8), ch(l, 3), Alu.min)
                tt(a1[:], ch(p, 2), ch(p, 3), Alu.mult)
                tt(a2[:], ch(p, 7), ch(p, 8), Alu.mult)
                tt(ag[:], ch(l, 2), ch(l, 3), Alu.mult)
                v.drain()
                # --- wave 2 ---
                ts(adx1[:].bitcast(U32), dx1[:].bitcast(U32), 0x7FFFFFFF, None,
                   Alu.bitwise_and)
                ts(ady1[:].bitcast(U32), dy1[:].bitcast(U32), 0x7FFFFFFF, None,
                   Alu.bitwise_and)
                ts(adx2[:].bitcast(U32), dx2[:].bitcast(U32), 0x7FFFFFFF, None,
                   Alu.bitwise_and)
                ts(ady2[:].bitcast(U32), dy2[:].bitcast(U32), 0x7FFFFFFF, None,
                   Alu.bitwise_and)
                tt(s1[:], a1[:], ag[:], Alu.add)
                tt(s2[:], a2[:], ag[:], Alu.add)
                v.drain()
                # --- wave 3: overlap = min(S-|d|, 7wa, 7wb), clamped ---
                stt(ta1[:], sw1[:], 3.5, adx1[:], Alu.mult, Alu.subtract)
                stt(ta2[:], sh1[:], 3.5, ady1[:], Alu.mult, Alu.subtract)
                stt(ta3[:], sw2[:], 3.5, adx2[:], Alu.mult, Alu.subtract)
                stt(ta4[:], sh2[:], 3.5, ady2[:], Alu.mult, Alu.subtract)
                v.drain()
                # --- wave 4: min vs 7*min(wa,wb) ---
                stt(tb1[:], tc1[:], 7.0, ta1[:], Alu.mult, Alu.min)
                stt(tb2[:], tc2[:], 7.0, ta2[:], Alu.mult, Alu.min)
                stt(tb3[:], tc3[:], 7.0, ta3[:], Alu.mult, Alu.min)
                stt(tb4[:], tc4[:], 7.0, ta4[:], Alu.mult, Alu.min)
                v.drain()
                # --- wave 5: clamp ---
                ts(iw1[:], tb1[:], 0.0, None, Alu.max)
                ts(ih1[:], tb2[:], 0.0, None, Alu.max)
                ts(iw2[:], tb3[:], 0.0, None, Alu.max)
                ts(ih2[:], tb4[:], 0.0, None, Alu.max)
                v.drain()
                # --- wave 7 ---
                tt(int1[:], iw1[:], ih1[:], Alu.mult)
                tt(int2[:], iw2[:], ih2[:], Alu.mult)
                v.drain()
                # --- wave 8: union = 49*(area_p + area_g) - inter ---
                stt(u1[:], s1[:], 49.0, int1[:], Alu.mult, Alu.subtract)
                stt(u2[:], s2[:], 49.0, int2[:], Alu.mult, Alu.subtract)
                v.drain()
                # --- wave 9 ---
                v.reciprocal(r1[:], u1[:])
                v.reciprocal(r2[:], u2[:])
                v.drain()
                # --- wave 10 ---
                tt(iou1[:], int1[:], r1[:], Alu.mult)
                tt(iou2[:], int2[:], r2[:], Alu.mult)
                v.drain()
                # --- wave 11 ---
                tt(use1[:], iou1[:], iou2[:], Alu.is_ge)
                tt(d1[:], ch(p, 4), iou1[:], Alu.subtract)
                tt(d2[:], ch(p, 9), iou2[:], Alu.subtract)
                v.drain().then_inc(sD1, 1)
                # --- wave 12: sqrt diffs (needs ACT phase 1) ---
                v.wait_ge(sA1, i + 1)
                tt(dsw1[:], sp2[:], sl2[:], Alu.subtract)
                tt(dsh1[:], sp3[:], sl3[:], Alu.subtract)
                tt(dsw2[:], sp7[:], sl7[:], Alu.subtract)
                tt(dsh2[:], sp8[:], sl8[:], Alu.subtract)
                v.drain().then_inc(sD2, 1)
                # --- wave 13+: combine (needs ACT phase 2) ---
                v.wait_ge(sA2, i + 1)
                v.tensor_reduce(
                    out=clsum[:],
                    in_=sqcls[:].rearrange("p g (c k) -> p g k c", c=20),
                    axis=mybir.AxisListType.X, op=Alu.add,
                )
                tt(de[:], e1[:], e2[:], Alu.subtract)
                tt(nc2t[:], q4[:], q9[:], Alu.add)
                tt(c1a[:], qx1[:], qy1[:], Alu.add)
                tt(c1b[:], qsw1[:], qsh1[:], Alu.add)
                tt(c2a[:], qx2[:], qy2[:], Alu.add)
                tt(c2b[:], qsw2[:], qsh2[:], Alu.add)
                v.drain()
                tt(coor1[:], c1a[:], c1b[:], Alu.add)
                tt(coor2[:], c2a[:], c2b[:], Alu.add)
                ts(hde[:], de[:], 0.5, None, Alu.mult)
                v.drain()
                tt(dc[:], coor1[:], coor2[:], Alu.subtract)
                stt(base[:], coor2[:], 5.0, e2[:], Alu.mult, Alu.add)
                v.drain()
                stt(mix[:], dc[:], 5.0, hde[:], Alu.mult, Alu.add)
                stt(base2[:], e1[:], 0.5, base[:], Alu.mult, Alu.add)
                v.drain()
                tt(tsel[:], use1[:], mix[:], Alu.mult)
                tt(junk2[:], clsum[:], base2[:], Alu.add)
                v.drain()
                tt(base3[:], junk2[:], tsel[:], Alu.add)
                v.drain()
                stt(dd[:], nc2t[:], -0.5, base3[:], Alu.mult, Alu.add)
                v.drain()
                # accumulate: acc0 += sum(obj * dd); acc1 += 0.5*sum(nc2)
                tt(junk[:], objm[:], dd[:], Alu.mult)
                v.drain()
                v.tensor_reduce(out=red0[:], in_=junk[:],
                                axis=mybir.AxisListType.XY, op=Alu.add)
                v.tensor_reduce(out=red1[:], in_=nc2t[:],
                                axis=mybir.AxisListType.XY, op=Alu.add)
                v.drain()
                stt(acc[:, 0:1], red0[:], 1.0, acc[:, 0:1], Alu.mult, Alu.add)
                stt(acc[:, 1:2], red1[:], 0.5, acc[:, 1:2], Alu.mult, Alu.add)
                v.drain().then_inc(v_done, 1)

    return nc


_NC_CACHE = {}


def _get_nc():
    if "nc" not in _NC_CACHE:
        _NC_CACHE["nc"] = build_nc()
    return _NC_CACHE["nc"]


def run_device(pred, labels, trace=False):
    nc = _get_nc()
    pred = np.ascontiguousarray(pred, dtype=np.float32).reshape(B_TOTAL, ROW)
    labels = np.ascontiguousarray(labels, dtype=np.float32).reshape(B_TOTAL, ROW)
    in_maps = []
    for c in range(NCORES):
        rows = slice(c * B_CORE, (c + 1) * B_CORE)
        in_maps.append({"pred": pred[rows], "labels": labels[rows]})
    res = run_bass_kernel_spmd(nc, in_maps, list(range(NCORES)), trace=trace)
    total = 0.0
    for c in range(NCORES):
        total += float(res.results[c]["out"][:, :3].astype(np.float64).sum())
    loss = np.float32(total / B_TOTAL)
    return loss, res


def kernel(pred, labels):
    loss, _ = run_device(pred, labels, trace=False)
    return np.array(loss, dtype=np.float32)


if __name__ == "__main__":
    rng = np.random.default_rng(0)
    p = rng.random((B_TOTAL, C, 7, 7), dtype=np.float32)
    l = rng.random((B_TOTAL, C, 7, 7), dtype=np.float32)
    l[:, 4] = (rng.random((B_TOTAL, 7, 7)) < 0.3).astype(np.float32)
    print(kernel(p, l))



# revision 8
# speedup vs baseline: 1.6076x; 1.6076x over previous
"""YOLOv1 loss kernel for Trainium2, 8-core data-parallel.

Strategy: shard batch (8192) across 8 cores (1024 rows each). Each core
streams its shard in NCHUNK chunks of g*128 rows laid out as
[128 partitions, g, channels*49] in SBUF. Channel-pair arithmetic uses
strided "pair views" (stride 245 = 5 channels) and stride-0 broadcast
APs so each instruction covers 2-4 channels of both IoU boxes at once.

Engine split (per chunk):
  DVE  : box-center diffs, min-extents, overlap stt chain, IoU
         (reciprocal), per-cell combines, channel reduces, stt-accum
         final sums (23 ops).
  ACT  : all squares/sqrt/abs/relu (9 ops), incl. the 20-channel class
         square.
  Pool : bulk subtract/add/mult streams (class diff, sqrt-diffs, sums,
         areas) + is_equal masks.
  DMA  : 4 transfers per chunk (pred/labels split into box channels
         0..9 and class channels 10..29; labels ch9 is unused and
         skipped).

IoU in cell-scaled coords: all boxes of a cell share the (+m,+n)/7
offset, so with half-extents 3.5*w the overlap along x is
max(0, min(3.5*(wa+wg) - |dcx|, 7*min(wa,wg))) in units of 1/7 cell;
inter49 = ovx*ovy, union49 = 49*(wa*ha + wg*hg) - inter49.

Loss decomposition (per cell, u = [iou1>=iou2], obj = [l4==1]):
  per_cell = 0.5*nc2 + obj*(t4 + 5*u*(dcoor + 0.1*de))
  t4   = 5*coor2 + e2 + 0.5*e1 + cls - 0.5*nc2
  nc2  = p4^2 + p9^2, e_i = (conf_i - iou_i)^2, de = e1-e2,
  dcoor = coor1-coor2.
Three per-partition accumulators per chunk via scalar_tensor_tensor
accum_out; host sums 8 cores x 128 partitions x NCHUNK x 3 and divides
by B.
"""

import sys

import numpy as np

for _p in ("/opt/trn_rl_repo", "/root/.axon_site/_ro/trn_rl_repo"):
    if _p not in sys.path:
        sys.path.insert(0, _p)

import concourse.bass as bass
import concourse.mybir as mybir
from concourse.ap import AP
from concourse.bass_utils import run_bass_kernel_spmd

F32 = mybir.dt.float32
Alu = mybir.AluOpType
Act = mybir.ActivationFunctionType

B_TOTAL = 8192
NCORES = 8
B_CORE = B_TOTAL // NCORES  # 1024
P = 128
K = 49  # cells
C = 30

CHUNKS = (2, 2, 2, 2)  # g per chunk; sum * 128 == B_CORE
GMAX = max(CHUNKS)
NCHUNK = len(CHUNKS)

PT_W = 10 * K   # pred channels 0..9
LT_W = 9 * K    # label channels 0..8 (ch9 unused)
CL_W = 20 * K   # channels 10..29


def pview(tile, offset, dims):
    """Strided free-dim view of an SBUF tile, keeping its partition entry."""
    base = tile[:]
    return AP(base.tensor, offset, [list(base.ap[0])] + [list(d) for d in dims])


def build_nc(chunks=CHUNKS):
    nchunk = len(chunks)
    assert sum(chunks) * P == B_CORE
    nc = bass.Bass()
    pred = nc.declare_dram_parameter("pred", [B_CORE, C * K], F32, isOutput=False)
    labels = nc.declare_dram_parameter("labels", [B_CORE, C * K], F32, isOutput=False)
    out = nc.declare_dram_parameter("out", [P, nchunk * 3], F32, isOutput=True)

    from contextlib import ExitStack

    ctx = ExitStack()
    with ctx:
        def sb(name, shape):
            return ctx.enter_context(nc.sbuf_tensor(name, shape, F32))

        # inputs, double buffered
        pt = [sb(f"pt{b}", [P, GMAX, PT_W]) for b in range(2)]
        lt = [sb(f"lt{b}", [P, GMAX, LT_W]) for b in range(2)]
        pc = [sb(f"pc{b}", [P, GMAX, CL_W]) for b in range(2)]
        lc = [sb(f"lc{b}", [P, GMAX, CL_W]) for b in range(2)]
        # scratch, double buffered
        D = [sb(f"D{b}", [P, GMAX, 6, K]) for b in range(2)]      # dxy1|dxy2|dxyc2
        AbsT = [sb(f"Ab{b}", [P, GMAX, 4, K]) for b in range(2)]
        S = [sb(f"S{b}", [P, GMAX, 4, K]) for b in range(2)]
        M = [sb(f"M{b}", [P, GMAX, 4, K]) for b in range(2)]
        T = [sb(f"T{b}", [P, GMAX, 4, K]) for b in range(2)]
        OV = [sb(f"OV{b}", [P, GMAX, 4, K]) for b in range(2)]
        OVC = [sb(f"OVC{b}", [P, GMAX, 4, K]) for b in range(2)]
        INT = [sb(f"INT{b}", [P, GMAX, 2, K]) for b in range(2)]
        AP12 = [sb(f"AP12{b}", [P, GMAX, 2, K]) for b in range(2)]
        AG = [sb(f"AG{b}", [P, GMAX, K]) for b in range(2)]
        U1 = [sb(f"U1{b}", [P, GMAX, 2, K]) for b in range(2)]
        UN = [sb(f"UN{b}", [P, GMAX, 2, K]) for b in range(2)]
        RC = [sb(f"RC{b}", [P, GMAX, 2, K]) for b in range(2)]
        IOU = [sb(f"IOU{b}", [P, GMAX, 2, K]) for b in range(2)]
        USE1 = [sb(f"USE1{b}", [P, GMAX, K]) for b in range(2)]
        MK2 = [sb(f"MK2{b}", [P, GMAX, K]) for b in range(2)]
        DCF = [sb(f"DCF{b}", [P, GMAX, 2, K]) for b in range(2)]
        E = [sb(f"E{b}", [P, GMAX, 2, K]) for b in range(2)]
        AB2 = [sb(f"AB2{b}", [P, GMAX, 2, K]) for b in range(2)]
        SP = [sb(f"SP{b}", [P, GMAX, 4, K]) for b in range(2)]
        SL = [sb(f"SL{b}", [P, GMAX, 4, K]) for b in range(2)]
        DSQ = [sb(f"DSQ{b}", [P, GMAX, 4, K]) for b in range(2)]
        Q = [sb(f"Q{b}", [P, GMAX, 2, 4, K]) for b in range(2)]
        CB = [sb(f"CB{b}", [P, GMAX, 2, K]) for b in range(2)]
        OBJ = [sb(f"OBJ{b}", [P, GMAX, K]) for b in range(2)]
        NC2 = [sb(f"NC2{b}", [P, GMAX, K]) for b in range(2)]
        CLS = [sb(f"CLS{b}", [P, GMAX, K]) for b in range(2)]
        DE = [sb(f"DE{b}", [P, GMAX, K]) for b in range(2)]
        DCO = [sb(f"DCO{b}", [P, GMAX, K]) for b in range(2)]
        BASE = [sb(f"BASE{b}", [P, GMAX, K]) for b in range(2)]
        T2 = [sb(f"T2{b}", [P, GMAX, K]) for b in range(2)]
        T5 = [sb(f"T5{b}", [P, GMAX, K]) for b in range(2)]
        T4 = [sb(f"T4{b}", [P, GMAX, K]) for b in range(2)]
        SEL = [sb(f"SEL{b}", [P, GMAX, K]) for b in range(2)]
        JA = [sb(f"JA{b}", [P, GMAX, K]) for b in range(2)]
        JB = [sb(f"JB{b}", [P, GMAX, K]) for b in range(2)]
        # single-buffered big class scratch
        DCLS = sb("DCLS", [P, GMAX, 20, K])
        SQC = sb("SQC", [P, GMAX, 20, K])
        acc = sb("acc", [P, nchunk * 3])

        dma_sem = ctx.enter_context(nc.semaphore("dma_sem"))
        sV1 = ctx.enter_context(nc.semaphore("sV1"))
        sV2 = ctx.enter_context(nc.semaphore("sV2"))
        sV3 = ctx.enter_context(nc.semaphore("sV3"))
        v_done = ctx.enter_context(nc.semaphore("v_done"))
        sA1 = ctx.enter_context(nc.semaphore("sA1"))
        sA2 = ctx.enter_context(nc.semaphore("sA2"))
        sA3 = ctx.enter_context(nc.semaphore("sA3"))
        sA4 = ctx.enter_context(nc.semaphore("sA4"))
        sA5 = ctx.enter_context(nc.semaphore("sA5"))
        sA6 = ctx.enter_context(nc.semaphore("sA6"))
        sP1 = ctx.enter_context(nc.semaphore("sP1"))
        sP2 = ctx.enter_context(nc.semaphore("sP2"))
        sPD = ctx.enter_context(nc.semaphore("sPD"))
        sP3 = ctx.enter_context(nc.semaphore("sP3"))
        block = ctx.enter_context(nc.Block())

        row_of = [0]
        for g in chunks:
            row_of.append(row_of[-1] + g * P)

        @block.sync
        def _(sync):
            for i, g in enumerate(chunks):
                b = i % 2
                if i >= 2:
                    sync.wait_ge(v_done, i - 1)
                    sync.wait_ge(sA6, i - 1)
                    sync.wait_ge(sP3, i - 1)
                rows = slice(row_of[i], row_of[i + 1])
                sync.dma_start(
                    out=pt[b][:, :g, :],
                    in_=pred[rows, 0:PT_W].rearrange("(g p) d -> p g d", p=P),
                ).then_inc(dma_sem, 16)
                sync.dma_start(
                    out=lt[b][:, :g, :],
                    in_=labels[rows, 0:LT_W].rearrange("(g p) d -> p g d", p=P),
                ).then_inc(dma_sem, 16)
                sync.dma_start(
                    out=pc[b][:, :g, :],
                    in_=pred[rows, PT_W:C * K].rearrange("(g p) d -> p g d", p=P),
                ).then_inc(dma_sem, 16)
                sync.dma_start(
                    out=lc[b][:, :g, :],
                    in_=labels[rows, PT_W:C * K].rearrange("(g p) d -> p g d", p=P),
                ).then_inc(dma_sem, 16)
            sync.wait_ge(v_done, nchunk)
            sync.dma_start(out=out[:], in_=acc[:]).then_inc(dma_sem, 16)
            sync.wait_ge(dma_sem, 64 * nchunk + 16)

        @block.gpsimd
        def _(gp):
            for i, g in enumerate(chunks):
                b = i % 2
                gp.wait_ge(dma_sem, 64 * i + 32)
                if i >= 2:
                    gp.wait_ge(v_done, i - 1)
                    gp.wait_ge(sA6, i - 1)
                p, l = pt[b], lt[b]
                pch = p[:, :g, :].rearrange("p g (c k) -> p g c k", c=10)
                lch = l[:, :g, :].rearrange("p g (c k) -> p g c k", c=9)
                # S = [pw1+lw, ph1+lh, pw2+lw, ph2+lh]
                def fl(ap4):  # [p,g,c,k] -> [p,g,(c k)] for 3-dim pool ops
                    return ap4.rearrange("p g c k -> p g (c k)")

                gp.tensor_tensor(fl(S[b][:, :g, 0:2, :]), fl(pch[:, :, 2:4, :]),
                                 fl(lch[:, :, 2:4, :]), Alu.add)
                gp.tensor_tensor(fl(S[b][:, :g, 2:4, :]), fl(pch[:, :, 7:9, :]),
                                 fl(lch[:, :, 2:4, :]), Alu.add)
                # dxyc2 = p(5,6) - l(5,6)  (coor2 xy diffs)
                gp.tensor_tensor(fl(D[b][:, :g, 4:6, :]), fl(pch[:, :, 5:7, :]),
                                 fl(lch[:, :, 5:7, :]), Alu.subtract)
                # objm = (l4 == 1)
                gp.tensor_scalar(OBJ[b][:, :g, :], lch[:, :, 4, :], 1.0, None,
                                 Alu.is_equal)
                # areas: ap12 = [pw1*ph1, pw2*ph2]; ag = lw*lh
                gp.tensor_tensor(AP12[b][:, :g, :, :], pch[:, :, 2:8:5, :],
                                 pch[:, :, 3:9:5, :], Alu.mult)
                gp.tensor_tensor(AG[b][:, :g, :], lch[:, :, 2, :],
                                 lch[:, :, 3, :], Alu.mult)
                gp.drain().then_inc(sP1, 1)
                # dsq = sqrt(p) - sqrt(l) for w/h channels of both boxes
                gp.wait_ge(sA1, i + 1)
                gp.tensor_tensor(DSQ[b][:, :g, :, :], SP[b][:, :g, :, :],
                                 SL[b][:, :g, :, :], Alu.subtract)
                gp.drain().then_inc(sP2, 1)
                # class diff (20 channels)
                gp.wait_ge(dma_sem, 64 * (i + 1))
                if i >= 1:
                    gp.wait_ge(sA6, i)  # SQC(i-1) consumed from DCLS
                gp.tensor_tensor(DCLS[:, :g, :, :].rearrange("p g c k -> p (g c) k"),
                                 pc[b][:, :g, :].rearrange("p g (c k) -> p (g c) k", c=20),
                                 lc[b][:, :g, :].rearrange("p g (c k) -> p (g c) k", c=20),
                                 Alu.subtract)
                gp.drain().then_inc(sPD, 1)
                # mask2 = obj * use1
                gp.wait_ge(sV3, i + 1)
                gp.tensor_tensor(MK2[b][:, :g, :], OBJ[b][:, :g, :],
                                 USE1[b][:, :g, :], Alu.mult)
                gp.drain().then_inc(sP3, 1)

        @block.scalar
        def _(act):
            for i, g in enumerate(chunks):
                b = i % 2
                act.wait_ge(dma_sem, 64 * i + 32)
                if i >= 2:
                    act.wait_ge(sP2, i - 1)
                    act.wait_ge(v_done, i - 1)
                p, l = pt[b], lt[b]
                pch = p[:, :g, :].rearrange("p g (c k) -> p g c k", c=10)
                # sqrt of w/h channels, pair views {2,3},{7,8}
                p_wh = pview(p, 2 * K, [[PT_W, g], [5 * K, 2], [K, 2], [1, K]])
                l_wh = pview(l, 2 * K, [[LT_W, g], [5 * K, 2], [K, 2], [1, K]])
                sp_o = SP[b][:, :g, :, :].rearrange("p g (pr c) k -> p g pr c k", pr=2)
                sl_o = SL[b][:, :g, :, :].rearrange("p g (pr c) k -> p g pr c k", pr=2)
                act.activation(sp_o, p_wh, Act.Sqrt)
                act.activation(sl_o, l_wh, Act.Sqrt)
                # conf squares A=p4^2, B=p9^2
                act.activation(AB2[b][:, :g, :, :], pch[:, :, 4:10:5, :], Act.Square)
                act.drain().then_inc(sA1, 1)
                # |d| of the four IoU center diffs
                act.wait_ge(sV1, i + 1)
                act.activation(AbsT[b][:, :g, :, :], D[b][:, :g, 0:4, :], Act.Abs)
                act.drain().then_inc(sA2, 1)
                # xy squares into Q slots [pair][0:2]
                act.wait_ge(sP1, i + 1)
                qxy_in = pview(D[b], 0, [[6 * K, g], [4 * K, 2], [K, 2], [1, K]])
                qxy_o = pview(Q[b], 0, [[8 * K, g], [4 * K, 2], [K, 2], [1, K]])
                act.activation(qxy_o, qxy_in, Act.Square)
                # relu of overlaps
                act.wait_ge(sV2, i + 1)
                act.activation(OVC[b][:, :g, :, :], OV[b][:, :g, :, :], Act.Relu)
                act.drain().then_inc(sA3, 1)
                # sqrt-diff squares into Q slots [pair][2:4]
                act.wait_ge(sP2, i + 1)
                qw_in = DSQ[b][:, :g, :, :].rearrange("p g (pr c) k -> p g pr c k", pr=2)
                qw_o = pview(Q[b], 2 * K, [[8 * K, g], [4 * K, 2], [K, 2], [1, K]])
                act.activation(qw_o, qw_in, Act.Square)
                act.drain().then_inc(sA4, 1)
                # conf err squares
                act.wait_ge(sV3, i + 1)
                act.activation(E[b][:, :g, :, :], DCF[b][:, :g, :, :], Act.Square)
                act.drain().then_inc(sA5, 1)
                # class squares (SQC is single-buffered: wait clsred(i-1))
                act.wait_ge(sPD, i + 1)
                if i >= 1:
                    act.wait_ge(v_done, i)
                act.activation(SQC[:, :g, :, :].rearrange("p g c k -> p (g c) k"),
                               DCLS[:, :g, :, :].rearrange("p g c k -> p (g c) k"),
                               Act.Square)
                act.drain().then_inc(sA6, 1)

        @block.vector
        def _(v):
            stt = v.scalar_tensor_tensor
            tt = v.tensor_tensor
            for i, g in enumerate(chunks):
                b = i % 2
                v.wait_ge(dma_sem, 64 * i + 32)
                if i >= 2:
                    v.wait_ge(sA3, i - 1)
                    v.wait_ge(sA5, i - 1)
                    v.wait_ge(sP3, i - 1)
                p, l = pt[b], lt[b]
                pch = p[:, :g, :].rearrange("p g (c k) -> p g c k", c=10)
                lch = l[:, :g, :].rearrange("p g (c k) -> p g c k", c=9)
                # w1: all four IoU center diffs in one op (l xy broadcast)
                p_xy = pview(p, 0, [[PT_W, g], [5 * K, 2], [K, 2], [1, K]])
                l_xy = lch[:, :, 0:2, :].unsqueeze(2).broadcast_to((P, g, 2, 2, K))
                d_o = D[b][:, :g, 0:4, :].rearrange("p g (pr c) k -> p g pr c k", pr=2)
                tt(d_o, p_xy, l_xy, Alu.subtract).then_inc(sV1, 1)
                # w2: min extents, t = 3.5*S - |d|, ov = min(7*M, t)
                p_wh = pview(p, 2 * K, [[PT_W, g], [5 * K, 2], [K, 2], [1, K]])
                l_wh_b = lch[:, :, 2:4, :].unsqueeze(2).broadcast_to((P, g, 2, 2, K))
                m_o = M[b][:, :g, :, :].rearrange("p g (pr c) k -> p g pr c k", pr=2)
                tt(m_o, p_wh, l_wh_b, Alu.min)
                v.wait_ge(sA2, i + 1)

                def fl(ap4):  # contiguous [p,g,c,k] -> 3D for stt/reciprocal
                    return ap4.rearrange("p g c k -> p g (c k)")

                stt(fl(T[b][:, :g, :, :]), fl(S[b][:, :g, :, :]), 3.5,
                    fl(AbsT[b][:, :g, :, :]), Alu.mult, Alu.subtract)
                stt(fl(OV[b][:, :g, :, :]), fl(M[b][:, :g, :, :]), 7.0,
                    fl(T[b][:, :g, :, :]), Alu.mult, Alu.min).then_inc(sV2, 1)
                # w3: inter, union, iou, use1, dconf
                v.wait_ge(sA3, i + 1)
                ovc = OVC[b][:, :g, :, :].rearrange("p g (pr c) k -> p g pr c k", pr=2)
                tt(INT[b][:, :g, :, :], ovc[:, :, :, 0, :], ovc[:, :, :, 1, :],
                   Alu.mult)
                v.wait_ge(sP1, i + 1)
                stt(fl(U1[b][:, :g, :, :]), fl(AP12[b][:, :g, :, :]), 49.0,
                    fl(INT[b][:, :g, :, :]), Alu.mult, Alu.subtract)
                for pr in range(2):
                    stt(UN[b][:, :g, pr, :], AG[b][:, :g, :], 49.0,
                        U1[b][:, :g, pr, :], Alu.mult, Alu.add)
                v.reciprocal(fl(RC[b][:, :g, :, :]), fl(UN[b][:, :g, :, :]))
                tt(IOU[b][:, :g, :, :], INT[b][:, :g, :, :], RC[b][:, :g, :, :],
                   Alu.mult)
                iouv = IOU[b][:, :g, :, :]
                tt(USE1[b][:, :g, :], iouv[:, :, 0, :], iouv[:, :, 1, :], Alu.is_ge)
                tt(DCF[b][:, :g, :, :], pch[:, :, 4:10:5, :], iouv, Alu.subtract)
                v.drain().then_inc(sV3, 1)
                # w4: reduces + combine + accumulate
                v.wait_ge(sA4, i + 1)
                # (g, pair) merge: pair stride 4K, g stride 8K -> uniform 4K
                q_in = pview(Q[b], 0, [[4 * K, 2 * g], [K, 4], [1, K]]).transpose(
                    [0, 1, 3, 2])
                cb_o = CB[b][:, :g, :, :].rearrange("p g c k -> p (g c) k")
                v.tensor_reduce(out=cb_o, in_=q_in, axis=mybir.AxisListType.X,
                                op=Alu.add)
                v.wait_ge(sA1, i + 1)
                ab2 = AB2[b][:, :g, :, :]
                stt(NC2[b][:, :g, :], ab2[:, :, 0, :], 1.0, ab2[:, :, 1, :],
                    Alu.mult, Alu.add, accum_out=acc[:, 3 * i + 0:3 * i + 1])
                v.wait_ge(sA5, i + 1)
                ev = E[b][:, :g, :, :]
                cbv = CB[b][:, :g, :, :]
                tt(DE[b][:, :g, :], ev[:, :, 0, :], ev[:, :, 1, :], Alu.subtract)
                tt(DCO[b][:, :g, :], cbv[:, :, 0, :], cbv[:, :, 1, :], Alu.subtract)
                stt(BASE[b][:, :g, :], cbv[:, :, 1, :], 5.0, ev[:, :, 1, :],
                    Alu.mult, Alu.add)
                stt(T2[b][:, :g, :], ev[:, :, 0, :], 0.5, BASE[b][:, :g, :],
                    Alu.mult, Alu.add)
                stt(SEL[b][:, :g, :], DE[b][:, :g, :], 0.1, DCO[b][:, :g, :],
                    Alu.mult, Alu.add)
                v.wait_ge(sA6, i + 1)
                cls_in = SQC[:, :g, :, :].transpose([0, 1, 3, 2])
                v.tensor_reduce(out=CLS[b][:, :g, :], in_=cls_in,
                                axis=mybir.AxisListType.X, op=Alu.add)
                tt(T5[b][:, :g, :], T2[b][:, :g, :], CLS[b][:, :g, :], Alu.add)
                stt(T4[b][:, :g, :], NC2[b][:, :g, :], -0.5, T5[b][:, :g, :],
                    Alu.mult, Alu.add)
                stt(JA[b][:, :g, :], OBJ[b][:, :g, :], 1.0, T4[b][:, :g, :],
                    Alu.mult, Alu.mult, accum_out=acc[:, 3 * i + 1:3 * i + 2])
                v.wait_ge(sP3, i + 1)
                stt(JB[b][:, :g, :], MK2[b][:, :g, :], 5.0, SEL[b][:, :g, :],
                    Alu.mult, Alu.mult, accum_out=acc[:, 3 * i + 2:3 * i + 3])
                v.drain().then_inc(v_done, 1)

    return nc


_NC_CACHE = {}


def _get_nc():
    if "nc" not in _NC_CACHE:
        _NC_CACHE["nc"] = build_nc()
    return _NC_CACHE["nc"]


def run_device(pred, labels, trace=False):
    nc = _get_nc()
    pred = np.ascontiguousarray(pred, dtype=np.float32).reshape(B_TOTAL, C * K)
    labels = np.ascontiguousarray(labels, dtype=np.float32).reshape(B_TOTAL, C * K)
    in_maps = []
    for c in range(NCORES):
        rows = slice(c * B_CORE, (c + 1) * B_CORE)
        in_maps.append({"pred": pred[rows], "labels": labels[rows]})
    res = run_bass_kernel_spmd(nc, in_maps, list(range(NCORES)), trace=trace)
    total = 0.0
    for c in range(NCORES):
        arr = res.results[c]["out"].astype(np.float64).reshape(P, NCHUNK, 3)
        total += 0.5 * arr[:, :, 0].sum() + arr[:, :, 1].sum() + arr[:, :, 2].sum()
    loss = np.float32(total / B_TOTAL)
    return loss, res


def kernel(pred, labels):
    loss, _ = run_device(pred, labels, trace=False)
    return np.array(loss, dtype=np.float32)


if __name__ == "__main__":
    rng = np.random.default_rng(0)
    p = rng.random((B_TOTAL, C, 7, 7), dtype=np.float32)
    l = rng.random((B_TOTAL, C, 7, 7), dtype=np.float32)
    l[:, 4] = (rng.random((B_TOTAL, 7, 7)) < 0.3).astype(np.float32)
    print(kernel(p, l))


# revision 17
# speedup vs baseline: 1.7451x; 1.0855x over previous
"""YOLOv1 loss kernel for Trainium2, 8-core data-parallel.

Strategy: shard batch (8192) across 8 cores (1024 rows each). Each core
streams its shard in chunks of g*128 rows laid out as
[128 partitions, g, channels*49] in SBUF. Channel-pair arithmetic uses
strided "pair views" (stride 245 = 5 channels) and stride-0 broadcast
APs so each instruction covers 2-4 channels of both IoU boxes at once.

The four engines run a software-pipelined schedule with fixed stage
offsets (stage(chunk j) issued in iteration j+k), so every consumed
value is >= 1 iteration old and engines never stall on fresh data:

  iter i   DMA : chunk i (box channels, then class channels;
                 class-first on the last chunk to shorten the drain)
           Pool: dsq(i-1), dcls(i-1) x2, mask2(i-1), box-ops(i)
           ACT : qw(i-1), sqrt/conf(i), sqcls_a(i-1), e(i-1), abs(i),
                 sqcls_b(i-1), qxy(i)
           DVE : w2+w3(i-1), dxy(i), cls-tail(i-2), w4-front(i-1)

Loss decomposition (per cell, u = [iou1>=iou2], obj = [l4==1]):
  per_cell = 0.5*nc2 + obj*(t4 + 5*u*(dcoor + 0.1*de))
  t4   = 5*coor2 + e2 + 0.5*e1 + cls - 0.5*nc2
  nc2  = p4^2 + p9^2, e_i = (conf_i - iou_i)^2, de = e1-e2,
  dcoor = coor1-coor2.
IoU in cell-scaled coords: overlap_x = max(0, min(3.5*(wa+wg) - |dcx|,
7*min(wa,wg))); inter49 = ovx*ovy, union49 = 49*(wa*ha+wg*hg) - inter49.

Per-partition accumulators (3 per chunk) via scalar_tensor_tensor
accum_out; host sums 8 cores x 128 partitions x nchunk x 3, divides by B.
"""

import sys

import numpy as np

for _p in ("/opt/trn_rl_repo", "/root/.axon_site/_ro/trn_rl_repo"):
    if _p not in sys.path:
        sys.path.insert(0, _p)

import concourse.bass as bass
import concourse.mybir as mybir
from concourse.ap import AP
from concourse.bass_utils import run_bass_kernel_spmd

F32 = mybir.dt.float32
Alu = mybir.AluOpType
Act = mybir.ActivationFunctionType
BF16 = mybir.dt.bfloat16

B_TOTAL = 8192
NCORES = 8
B_CORE = B_TOTAL // NCORES  # 1024
P = 128
K = 49  # cells
C = 30

CHUNKS = (1, 2, 2, 2, 1)  # g per chunk; sum * 128 == B_CORE
NIB = 3  # input buffers

PT_W = 10 * K   # pred channels 0..9
LT_W = 9 * K    # label channels 0..8 (ch9 unused)
CL_W = 20 * K   # channels 10..29


def pview(tile, offset, dims):
    """Strided free-dim view of an SBUF tile, keeping its partition entry."""
    base = tile[:]
    return AP(base.tensor, offset, [list(base.ap[0])] + [list(d) for d in dims])


def fl(ap4):
    """Contiguous [p,g,c,k] -> [p,g,(c k)] (3D for stt/reciprocal/pool)."""
    return ap4.rearrange("p g c k -> p g (c k)")


def build_nc(chunks=CHUNKS):
    n = len(chunks)
    gmax = max(chunks)
    assert sum(chunks) * P == B_CORE
    nc = bass.Bass()
    pred = nc.declare_dram_parameter("pred", [B_CORE, C * K], F32, isOutput=False)
    labels = nc.declare_dram_parameter("labels", [B_CORE, C * K], F32, isOutput=False)
    out = nc.declare_dram_parameter("out", [P, n * 3], F32, isOutput=True)

    from contextlib import ExitStack

    ctx = ExitStack()
    with ctx:
        def sb(name, shape, dt=F32):
            return ctx.enter_context(nc.sbuf_tensor(name, shape, dt))

        def sb2(name, shape, dt=F32):
            return [sb(f"{name}{b}", shape, dt) for b in range(2)]

        def sb3(name, shape, dt=F32):
            return [sb(f"{name}{b}", shape, dt) for b in range(3)]

        # inputs, triple buffered
        pt = sb3("pt", [P, gmax, PT_W])
        lt = sb3("lt", [P, gmax, LT_W])
        pc = sb3("pc", [P, gmax, CL_W])
        lc = sb3("lc", [P, gmax, CL_W])
        # scratch, double buffered (parity = chunk % 2)
        D = sb2("D", [P, gmax, 6, K])       # dxy1 | dxy2 | dxyc2
        AbsT = sb2("Ab", [P, gmax, 4, K])
        S = sb2("S", [P, gmax, 4, K])
        M = sb2("M", [P, gmax, 4, K])
        T = sb2("T", [P, gmax, 4, K])
        OV = sb2("OV", [P, gmax, 4, K])
        OVC = sb2("OVC", [P, gmax, 4, K])
        INT = sb2("INT", [P, gmax, 2, K])
        AP12 = sb2("AP12", [P, gmax, 2, K])
        AG = sb2("AG", [P, gmax, K])
        U1 = sb2("U1", [P, gmax, 2, K])
        UN = sb2("UN", [P, gmax, 2, K])
        RC = sb2("RC", [P, gmax, 2, K])
        IOU = sb2("IOU", [P, gmax, 2, K])
        USE1 = sb2("USE1", [P, gmax, K])
        DCF = sb2("DCF", [P, gmax, 2, K])
        E = sb2("E", [P, gmax, 2, K])
        AB2 = sb2("AB2", [P, gmax, 2, K])
        SP = sb2("SP", [P, gmax, 4, K])
        SL = sb2("SL", [P, gmax, 4, K])
        DSQ = sb2("DSQ", [P, gmax, 4, K])
        Q = sb2("Q", [P, gmax, 2, 4, 50], BF16)   # cell dim padded to 50
        TQ = sb2("TQ", [P, gmax, 2, 100], BF16)
        T10 = sb2("T10", [P, gmax, 10, 50], BF16)
        T5C = sb2("T5C", [P, gmax, 5, 50], BF16)
        TA = sb2("TA", [P, gmax, 2, 50], BF16)
        TBB = sb2("TBB", [P, gmax, 50], BF16)
        CB = sb2("CB", [P, gmax, 2, K])
        NC2 = sb2("NC2", [P, gmax, K])
        CLA = sb2("CLA", [P, gmax, K])
        DE = sb2("DE", [P, gmax, K])
        DCO = sb2("DCO", [P, gmax, K])
        BASE = sb2("BASE", [P, gmax, K])
        T2 = sb2("T2", [P, gmax, K])
        T5 = sb2("T5", [P, gmax, K])
        T4 = sb2("T4", [P, gmax, K])
        SEL = sb2("SEL", [P, gmax, K])
        JA = sb2("JA", [P, gmax, K])
        JB = sb2("JB", [P, gmax, K])
        DCLS = sb2("DCLS", [P, gmax, 20, K])
        # triple buffered: written by Pool/ACT, read 2 iterations later
        OBJ = sb3("OBJ", [P, gmax, K])
        MK2 = sb3("MK2", [P, gmax, K])
        SQC = sb3("SQC", [P, gmax, 20, 50], BF16)
        acc = sb("acc", [P, n * 3])

        sems = {}
        for nm in ("dma_sem", "sV1", "sV3", "sV4", "v_done", "sA1", "sA2",
                   "sA3", "sA4", "sA5", "sA6", "sP1", "sP2", "sPD", "sP3"):
            sems[nm] = ctx.enter_context(nc.semaphore(nm))
        dma_sem = sems["dma_sem"]
        sV1, sV3, sV4, v_done = sems["sV1"], sems["sV3"], sems["sV4"], sems["v_done"]
        sA1, sA2, sA3 = sems["sA1"], sems["sA2"], sems["sA3"]
        sA4, sA5, sA6 = sems["sA4"], sems["sA5"], sems["sA6"]
        sP1, sP2, sPD, sP3 = sems["sP1"], sems["sP2"], sems["sPD"], sems["sP3"]
        block = ctx.enter_context(nc.Block())

        row_of = [0]
        for g in chunks:
            row_of.append(row_of[-1] + g * P)
        # dma_sem counts at which box / cls parts of chunk j are resident
        boxrdy = [64 * j + 32 if j < n - 1 else 64 * (j + 1) for j in range(n)]
        clsrdy = [64 * (j + 1) if j < n - 1 else 64 * j + 32 for j in range(n)]

        def w(eng, s, v):
            if v > 0:
                eng.wait_ge(s, v)

        @block.sync
        def _(sync):
            for j, g in enumerate(chunks):
                b = j % NIB
                if j >= NIB:
                    w(sync, sV3, j - 2)        # V_23(j-3) done with inputs
                    w(sync, sPD, 2 * (j - 2))  # dcls(j-3) done
                    w(sync, sA1, j - 2)        # A_0(j-3) done
                rows = slice(row_of[j], row_of[j + 1])
                box = [
                    (pt[b][:, :g, :], pred, 0, PT_W),
                    (lt[b][:, :g, :], labels, 0, LT_W),
                ]
                cls = [
                    (pc[b][:, :g, :], pred, PT_W, C * K),
                    (lc[b][:, :g, :], labels, PT_W, C * K),
                ]
                parts = box + cls if j < n - 1 else cls + box
                for o, srct, c0, c1 in parts:
                    sync.dma_start(
                        out=o,
                        in_=srct[rows, c0:c1].rearrange("(g p) d -> p g d", p=P),
                    ).then_inc(dma_sem, 16)
            sync.wait_ge(v_done, n)
            sync.dma_start(out=out[:], in_=acc[:]).then_inc(dma_sem, 16)
            sync.wait_ge(dma_sem, 64 * n + 16)

        @block.gpsimd
        def _(gp):
            for i in range(n + 2):
                if 1 <= i <= n:
                    j, s, g, b = i - 1, (i - 1) % 2, chunks[i - 1], (i - 1) % NIB
                    # dsq(j) = sqrt(p) - sqrt(l), w/h channels of both boxes
                    w(gp, sA1, j + 1)
                    w(gp, sA4, j - 1)  # qw(j-2) done with DSQ[s]
                    gp.tensor_tensor(fl(DSQ[s][:, :g, :, :]),
                                     fl(SP[s][:, :g, :, :]),
                                     fl(SL[s][:, :g, :, :]), Alu.subtract)
                    gp.drain().then_inc(sP2, 1)
                    # class diff of chunk j, two 10-channel halves
                    w(gp, dma_sem, clsrdy[j])
                    w(gp, sA6, 2 * (j - 1))  # sqcls(j-2) done with DCLS[s]
                    for h in range(2):
                        gp.tensor_tensor(
                            fl(DCLS[s][:, :g, 10 * h:10 * (h + 1), :]),
                            pc[b][:, :g, 490 * h:490 * (h + 1)],
                            lc[b][:, :g, 490 * h:490 * (h + 1)],
                            Alu.subtract)
                        gp.drain().then_inc(sPD, 1)
                    # mask2(j) = obj * use1
                    w(gp, sV3, j + 1)
                    w(gp, v_done, j - 2)  # V_T(j-3) done with MK2[j%3]
                    gp.tensor_tensor(MK2[j % NIB][:, :g, :],
                                     OBJ[j % NIB][:, :g, :],
                                     USE1[s][:, :g, :], Alu.mult)
                    gp.drain().then_inc(sP3, 1)
                if i < n:
                    j, s, g, b = i, i % 2, chunks[i], i % NIB
                    w(gp, dma_sem, boxrdy[j])
                    w(gp, sV3, j - 1)     # V_23(j-2) done with S/D/AP12/AG
                    w(gp, sA3, j - 1)     # qxy(j-2) done with D[4:6]
                    w(gp, v_done, j - 1)  # V_T(j-3) done with OBJ[j%3]
                    p, l = pt[b], lt[b]
                    pch = p[:, :g, :].rearrange("p g (c k) -> p g c k", c=10)
                    lch = l[:, :g, :].rearrange("p g (c k) -> p g c k", c=9)
                    # S = [pw1+lw, ph1+lh, pw2+lw, ph2+lh]
                    gp.tensor_tensor(fl(S[s][:, :g, 0:2, :]),
                                     fl(pch[:, :, 2:4, :]),
                                     fl(lch[:, :, 2:4, :]), Alu.add)
                    gp.tensor_tensor(fl(S[s][:, :g, 2:4, :]),
                                     fl(pch[:, :, 7:9, :]),
                                     fl(lch[:, :, 2:4, :]), Alu.add)
                    # dxyc2 = p(5,6) - l(5,6)
                    gp.tensor_tensor(fl(D[s][:, :g, 4:6, :]),
                                     fl(pch[:, :, 5:7, :]),
                                     fl(lch[:, :, 5:7, :]), Alu.subtract)
                    # objm = (l4 == 1)
                    gp.tensor_scalar(OBJ[j % NIB][:, :g, :], lch[:, :, 4, :],
                                     1.0, None, Alu.is_equal)
                    # areas: ap12 = [pw1*ph1, pw2*ph2]; ag = lw*lh
                    gp.tensor_tensor(AP12[s][:, :g, :, :], pch[:, :, 2:8:5, :],
                                     pch[:, :, 3:9:5, :], Alu.mult)
                    gp.tensor_tensor(AG[s][:, :g, :], lch[:, :, 2, :],
                                     lch[:, :, 3, :], Alu.mult)
                    gp.drain().then_inc(sP1, 1)

        @block.scalar
        def _(act):
            for i in range(n + 2):
                if 1 <= i <= n:
                    j, s, g = i - 1, (i - 1) % 2, chunks[i - 1]
                    # qw(j): sqrt-diff squares into Q slots [pair][2:4]
                    w(act, sP2, j + 1)
                    w(act, sV4, j - 1)  # V_4(j-2) done with Q[s]
                    qw_in = DSQ[s][:, :g, :, :].rearrange(
                        "p g (pr c) k -> p g pr c k", pr=2)
                    qw_o = pview(Q[s], 100,
                                 [[400, g], [200, 2], [50, 2], [1, K]])
                    act.activation(qw_o, qw_in, Act.Square)
                    act.drain().then_inc(sA4, 1)
                if i < n:
                    j, s, g, b = i, i % 2, chunks[i], i % NIB
                    # sqrt of w/h channels + conf squares of chunk j
                    w(act, dma_sem, boxrdy[j])
                    w(act, sP2, j - 1)  # dsq(j-2) done with SP/SL[s]
                    w(act, sV4, j - 1)  # V_4(j-2) done with AB2[s]
                    p, l = pt[b], lt[b]
                    pch = p[:, :g, :].rearrange("p g (c k) -> p g c k", c=10)
                    p_wh = pview(p, 2 * K, [[PT_W, g], [5 * K, 2], [K, 2], [1, K]])
                    l_wh = pview(l, 2 * K, [[LT_W, g], [5 * K, 2], [K, 2], [1, K]])
                    sp_o = SP[s][:, :g, :, :].rearrange(
                        "p g (pr c) k -> p g pr c k", pr=2)
                    sl_o = SL[s][:, :g, :, :].rearrange(
                        "p g (pr c) k -> p g pr c k", pr=2)
                    act.activation(sp_o, p_wh, Act.Sqrt)
                    act.activation(sl_o, l_wh, Act.Sqrt)
                    act.activation(AB2[s][:, :g, :, :], pch[:, :, 4:10:5, :],
                                   Act.Square)
                    act.drain().then_inc(sA1, 1)
                if 1 <= i <= n:
                    j, s, g = i - 1, (i - 1) % 2, chunks[i - 1]
                    # class squares half a of chunk j
                    w(act, sPD, 2 * j + 1)
                    w(act, v_done, j - 1)  # V_T(j-2) done with SQC[j%3]
                    act.activation(
                        SQC[j % NIB][:, :g, 0:10, 0:49],
                        DCLS[s][:, :g, 0:10, :], Act.Square)
                    act.drain().then_inc(sA6, 1)
                    # e(j) = (conf - iou)^2
                    w(act, sV3, j + 1)
                    w(act, sV4, j - 1)  # V_4(j-2) done with E[s]
                    act.activation(E[s][:, :g, :, :], DCF[s][:, :g, :, :],
                                   Act.Square)
                    act.drain().then_inc(sA5, 1)
                if i < n:
                    j, s, g = i, i % 2, chunks[i]
                    # |d| of the four IoU center diffs of chunk j
                    w(act, sV1, j + 1)
                    w(act, sV3, j - 1)  # V_23(j-2) done with AbsT[s]
                    act.activation(AbsT[s][:, :g, :, :], D[s][:, :g, 0:4, :],
                                   Act.Abs)
                    act.drain().then_inc(sA2, 1)
                if 1 <= i <= n:
                    j, s, g = i - 1, (i - 1) % 2, chunks[i - 1]
                    # class squares half b of chunk j
                    w(act, sPD, 2 * j + 2)
                    act.activation(
                        SQC[j % NIB][:, :g, 10:20, 0:49],
                        DCLS[s][:, :g, 10:20, :], Act.Square)
                    act.drain().then_inc(sA6, 1)
                if i < n:
                    j, s, g = i, i % 2, chunks[i]
                    # qxy(j): xy-diff squares into Q slots [pair][0:2]
                    w(act, sP1, j + 1)
                    w(act, sV4, j - 1)  # V_4(j-2) done with Q[s]
                    qxy_in = pview(D[s], 0,
                                   [[6 * K, g], [4 * K, 2], [K, 2], [1, K]])
                    qxy_o = pview(Q[s], 0,
                                  [[400, g], [200, 2], [50, 2], [1, K]])
                    act.activation(qxy_o, qxy_in, Act.Square)
                    act.drain().then_inc(sA3, 1)

        @block.vector
        def _(v):
            stt = v.scalar_tensor_tensor
            tt = v.tensor_tensor
            for i in range(n + 2):
                if 1 <= i <= n:
                    # w2+w3 of chunk j = i-1
                    j, s, g, b = i - 1, (i - 1) % 2, chunks[i - 1], (i - 1) % NIB
                    w(v, sA2, j + 1)
                    w(v, sP1, j + 1)
                    w(v, sP3, j - 1)  # mask2(j-2) done with USE1[s]
                    w(v, sA5, j - 1)  # e(j-2) done with DCF[s]
                    p, l = pt[b], lt[b]
                    pch = p[:, :g, :].rearrange("p g (c k) -> p g c k", c=10)
                    lch = l[:, :g, :].rearrange("p g (c k) -> p g c k", c=9)
                    p_wh = pview(p, 2 * K, [[PT_W, g], [5 * K, 2], [K, 2], [1, K]])
                    l_wh_b = lch[:, :, 2:4, :].unsqueeze(2).broadcast_to(
                        (P, g, 2, 2, K))
                    m_o = M[s][:, :g, :, :].rearrange(
                        "p g (pr c) k -> p g pr c k", pr=2)
                    tt(m_o, p_wh, l_wh_b, Alu.min)
                    stt(fl(T[s][:, :g, :, :]), fl(S[s][:, :g, :, :]), 3.5,
                        fl(AbsT[s][:, :g, :, :]), Alu.mult, Alu.subtract)
                    stt(fl(OV[s][:, :g, :, :]), fl(M[s][:, :g, :, :]), 7.0,
                        fl(T[s][:, :g, :, :]), Alu.mult, Alu.min)
                    v.tensor_scalar(fl(OVC[s][:, :g, :, :]),
                                    fl(OV[s][:, :g, :, :]), 0.0, None, Alu.max)
                    ovc = OVC[s][:, :g, :, :].rearrange(
                        "p g (pr c) k -> p g pr c k", pr=2)
                    tt(INT[s][:, :g, :, :], ovc[:, :, :, 0, :],
                       ovc[:, :, :, 1, :], Alu.mult)
                    stt(fl(U1[s][:, :g, :, :]), fl(AP12[s][:, :g, :, :]), 49.0,
                        fl(INT[s][:, :g, :, :]), Alu.mult, Alu.subtract)
                    for pr in range(2):
                        stt(UN[s][:, :g, pr, :], AG[s][:, :g, :], 49.0,
                            U1[s][:, :g, pr, :], Alu.mult, Alu.add)
                    v.reciprocal(fl(RC[s][:, :g, :, :]), fl(UN[s][:, :g, :, :]))
                    tt(IOU[s][:, :g, :, :], INT[s][:, :g, :, :],
                       RC[s][:, :g, :, :], Alu.mult)
                    iouv = IOU[s][:, :g, :, :]
                    tt(USE1[s][:, :g, :], iouv[:, :, 0, :], iouv[:, :, 1, :],
                       Alu.is_ge)
                    tt(DCF[s][:, :g, :, :], pch[:, :, 4:10:5, :], iouv,
                       Alu.subtract)
                    v.drain().then_inc(sV3, 1)
                if i < n:
                    # w1 (dxy) of chunk j = i
                    j, s, g, b = i, i % 2, chunks[i], i % NIB
                    w(v, dma_sem, boxrdy[j])
                    w(v, sA3, j - 1)  # qxy(j-2) done with D[s]
                    p, l = pt[b], lt[b]
                    lch = l[:, :g, :].rearrange("p g (c k) -> p g c k", c=9)
                    p_xy = pview(p, 0, [[PT_W, g], [5 * K, 2], [K, 2], [1, K]])
                    l_xy = lch[:, :, 0:2, :].unsqueeze(2).broadcast_to(
                        (P, g, 2, 2, K))
                    d_o = D[s][:, :g, 0:4, :].rearrange(
                        "p g (pr c) k -> p g pr c k", pr=2)
                    tt(d_o, p_xy, l_xy, Alu.subtract).then_inc(sV1, 1)
                if 2 <= i <= n + 1:
                    # class tail of chunk j = i-2: bf16 add-tree 20 -> 1
                    j, s, g = i - 2, (i - 2) % 2, chunks[i - 2]
                    sq = SQC[j % NIB]
                    w(v, sA6, 2 * j + 2)
                    tt(T10[s][:, :g, :, :].rearrange("p g c k -> p g (c k)"),
                       sq[:, :g, 0:10, :].rearrange("p g c k -> p g (c k)"),
                       sq[:, :g, 10:20, :].rearrange("p g c k -> p g (c k)"),
                       Alu.add)
                    tt(T5C[s][:, :g, :, :].rearrange("p g c k -> p g (c k)"),
                       T10[s][:, :g, 0:5, :].rearrange("p g c k -> p g (c k)"),
                       T10[s][:, :g, 5:10, :].rearrange("p g c k -> p g (c k)"),
                       Alu.add)
                    tt(TA[s][:, :g, :, :].rearrange("p g c k -> p g (c k)"),
                       T5C[s][:, :g, 0:2, :].rearrange("p g c k -> p g (c k)"),
                       T5C[s][:, :g, 2:4, :].rearrange("p g c k -> p g (c k)"),
                       Alu.add)
                    tt(TBB[s][:, :g, :], TA[s][:, :g, 0, :], TA[s][:, :g, 1, :],
                       Alu.add)
                    tt(CLA[s][:, :g, :], TBB[s][:, :g, 0:49],
                       T5C[s][:, :g, 4, 0:49], Alu.add)
                    tt(T5[s][:, :g, :], T2[s][:, :g, :], CLA[s][:, :g, :],
                       Alu.add)
                    stt(T4[s][:, :g, :], NC2[s][:, :g, :], -0.5,
                        T5[s][:, :g, :], Alu.mult, Alu.add)
                    stt(JA[s][:, :g, :], OBJ[j % NIB][:, :g, :], 1.0,
                        T4[s][:, :g, :], Alu.mult, Alu.mult,
                        accum_out=acc[:, 3 * j + 1:3 * j + 2])
                    w(v, sP3, j + 1)
                    stt(JB[s][:, :g, :], MK2[j % NIB][:, :g, :], 5.0,
                        SEL[s][:, :g, :], Alu.mult, Alu.mult,
                        accum_out=acc[:, 3 * j + 2:3 * j + 3])
                    v.drain().then_inc(v_done, 1)
                if 1 <= i <= n:
                    # w4 front of chunk j = i-1
                    j, s, g = i - 1, (i - 1) % 2, chunks[i - 1]
                    w(v, sA4, j + 1)
                    w(v, sA3, j + 1)
                    qv = Q[s][:, :g, :, :, :]
                    tt(TQ[s][:, :g, :, :],
                       qv[:, :, :, 0:2, :].rearrange("p g pr c k -> p g pr (c k)"),
                       qv[:, :, :, 2:4, :].rearrange("p g pr c k -> p g pr (c k)"),
                       Alu.add)
                    tt(CB[s][:, :g, :, :], TQ[s][:, :g, :, 0:49],
                       TQ[s][:, :g, :, 50:99], Alu.add)
                    w(v, sA1, j + 1)
                    ab2 = AB2[s][:, :g, :, :]
                    stt(NC2[s][:, :g, :], ab2[:, :, 0, :], 1.0, ab2[:, :, 1, :],
                        Alu.mult, Alu.add,
                        accum_out=acc[:, 3 * j + 0:3 * j + 1])
                    w(v, sA5, j + 1)
                    ev = E[s][:, :g, :, :]
                    cbv = CB[s][:, :g, :, :]
                    tt(DE[s][:, :g, :], ev[:, :, 0, :], ev[:, :, 1, :],
                       Alu.subtract)
                    tt(DCO[s][:, :g, :], cbv[:, :, 0, :], cbv[:, :, 1, :],
                       Alu.subtract)
                    stt(BASE[s][:, :g, :], cbv[:, :, 1, :], 5.0, ev[:, :, 1, :],
                        Alu.mult, Alu.add)
                    stt(T2[s][:, :g, :], ev[:, :, 0, :], 0.5, BASE[s][:, :g, :],
                        Alu.mult, Alu.add)
                    stt(SEL[s][:, :g, :], DE[s][:, :g, :], 0.1, DCO[s][:, :g, :],
                        Alu.mult, Alu.add)
                    v.drain().then_inc(sV4, 1)

    return nc


_NC_CACHE = {}


def _get_nc():
    if "nc" not in _NC_CACHE:
        _NC_CACHE["nc"] = build_nc()
    return _NC_CACHE["nc"]


def run_device(pred, labels, trace=False):
    nc = _get_nc()
    pred = np.ascontiguousarray(pred, dtype=np.float32).reshape(B_TOTAL, C * K)
    labels = np.ascontiguousarray(labels, dtype=np.float32).reshape(B_TOTAL, C * K)
    in_maps = []
    for c in range(NCORES):
        rows = slice(c * B_CORE, (c + 1) * B_CORE)
        in_maps.append({"pred": pred[rows], "labels": labels[rows]})
    res = run_bass_kernel_spmd(nc, in_maps, list(range(NCORES)), trace=trace)
    total = 0.0
    for c in range(NCORES):
        arr = res.results[c]["out"].astype(np.float64).reshape(P, len(CHUNKS), 3)
        total += 0.5 * arr[:, :, 0].sum() + arr[:, :, 1].sum() + arr[:, :, 2].sum()
    loss = np.float32(total / B_TOTAL)
    return loss, res


def kernel(pred, labels):
    loss, _ = run_device(pred, labels, trace=False)
    return np.array(loss, dtype=np.float32)


if __name__ == "__main__":
    rng = np.random.default_rng(0)
    p = rng.random((B_TOTAL, C, 7, 7), dtype=np.float32)
    l = rng.random((B_TOTAL, C, 7, 7), dtype=np.float32)
    l[:, 4] = (rng.random((B_TOTAL, 7, 7)) < 0.3).astype(np.float32)
    print(kernel(p, l))
